# revision 9
# baseline (speedup 1.0000x reference)
"""AdaptiveTokenSampling on 8 TRN2 NeuronCores (Bass/Tile, batch-parallel).

Per-core (one batch element):
  1. score pipeline: value norms + cls attention -> pseudo-logits (token-partition layout)
  2. gumbel argmax sampling (vector.max/max_index) -> 256 sampled token ids
  3. sort-based unique via comparison matrices (DVE) + positional scatter (PE matmuls)
  4. indirect-DMA gather of attn rows -> new_attn (4 SWDGE queues, 2 HWDGE write queues)
"""
import numpy as np

import concourse.bacc as bacc
import concourse.bass as bass
import concourse.mybir as mybir
import concourse.tile as tile
from concourse.bass_utils import run_bass_kernel_spmd

F32 = mybir.dt.float32
I32 = mybir.dt.int32
U32 = mybir.dt.uint32
U8 = mybir.dt.uint8

B, H, N, D, K = 8, 12, 1025, 64, 256
NM1 = N - 1          # 1024
NH = N * H           # 12300 rows in flattened attn
KP1 = K + 1          # 257
EPS = 1e-6
MASKVAL = float(-np.finfo(np.float32).max / 2)
DUMP = 300.0         # parking slot for non-survivors (maps outside valid q', contributes 0)
VCH = 3              # heads per value-pipeline chunk
ALU = mybir.AluOpType


def _indirect_gather_q(nc, out, in_, offset_ap, axis, qnum):
    """gpsimd.indirect_dma_start with a selectable SWDGE queue (0..3)."""
    eng = nc.gpsimd
    out_ap = eng.lower_ap_dma(out, for_indirect_dma=True)
    in_ap = eng.lower_ap_dma(in_, for_indirect_dma=True)
    assert len(in_ap) == 1 and len(out_ap) == 1
    off = eng.lower_ap_dma(offset_ap)
    assert len(off) == 1
    in_ap.append(off[0])
    coef = 1
    for i in range(axis + 1, len(in_.shape)):
        coef *= in_.shape[i]
    in_ap[0].dynamic_ap_info = mybir.DynamicAccessPatternInfo(
        c=0,
        actual_ap=out.ap,
        indirect_dim_max_index=in_.shape[axis],
        offset_expr=[
            mybir.DynamicAccessPatternOffsetExpr(
                coef=coef,
                aff_expr=mybir.DynamicAccessPatternOffsetExprAffExpr(
                    kind="IndirectArgId", arg_id=1,
                ),
            )
        ],
    )
    return eng.add_instruction(
        mybir.InstDMACopy(
            name=nc.get_next_instruction_name(),
            queue=f"qPoolDynamic{qnum or ''}",
            mode="Copy",
            ins=in_ap,
            outs=out_ap,
            oob_is_err=True,
            cce_op=ALU.bypass,
        )
    )


def _build():
    nc = bacc.Bacc(None, target_bir_lowering=False, debug=False, num_devices=8,
                   num_swdge_queues=4)

    attn_d = nc.declare_dram_parameter("attn", [NH, N], F32, isOutput=False)
    val_d = nc.declare_dram_parameter("value", [H, N * D], F32, isOutput=False)
    msk_d = nc.declare_dram_parameter("maskp", [N], U8, isOutput=False)
    gum_d = nc.declare_dram_parameter("gumbel", [K, NM1], F32, isOutput=False)

    oattn_d = nc.declare_dram_parameter("out_attn", [H, KP1, N], F32, isOutput=True)
    oids_d = nc.declare_dram_parameter("out_ids", [KP1], I32, isOutput=True)
    omask_d = nc.declare_dram_parameter("out_mask", [KP1], U8, isOutput=True)

    pl_dram = nc.dram_tensor("pl_dram", [NM1], F32)

    ident_c = nc.inline_tensor(np.eye(128, dtype=np.float32), name="ident_c")
    iota128_c = nc.inline_tensor(
        np.broadcast_to(np.arange(128, dtype=np.float32), (128, 128)).copy(), name="iota128_c")
    gidx = np.arange(K, dtype=np.float32)
    p128 = np.arange(128, dtype=np.float32)
    # ILT_a[p, i] = 1.0 if global index (p+128a) < i   (earlier-than-target, target on free)
    ilt_c = [nc.inline_tensor(((p128 + 128 * a)[:, None] < gidx[None, :]).astype(np.float32),
                              name=f"ilt{a}_c") for a in range(2)]
    # ILTrev_a[p, q] = 1.0 if q < (p+128a)             (earlier-than-target, target on partition)
    iltr_c = [nc.inline_tensor((gidx[None, :] < (p128 + 128 * a)[:, None]).astype(np.float32),
                               name=f"iltr{a}_c") for a in range(2)]
    # head offsets replicated for the [128, 2H] gather-index add: col 2h+c -> h*N
    hoffm_c = nc.inline_tensor(
        np.broadcast_to(np.repeat(np.arange(H, dtype=np.float32) * float(N), 2), (128, 2 * H)).copy(),
        name="hoffm_c")

    with tile.TileContext(nc) as tc:
        with (
            tc.tile_pool(name="const", bufs=1) as cp,
            tc.tile_pool(name="work", bufs=2) as wp,
            tc.tile_pool(name="keep", bufs=1) as kp,
            tc.tile_pool(name="ps", bufs=2, space="PSUM") as ps,
            tc.tile_pool(name="psacc", bufs=2, space="PSUM") as psa,
            tc.tile_pool(name="gath", bufs=8) as gp,
        ):
            # ---- tiny DVE constants + ACT table warmups (off critical path) ----
            ones_col = cp.tile([128, 1], F32)
            nc.vector.memset(ones_col[:], 1.0)
            ones_row = cp.tile([1, 128], F32)
            nc.vector.memset(ones_row[:], 1.0)
            one_cell = cp.tile([1, 1], F32)
            nc.vector.memset(one_cell[:], 1.0)
            eps_col = cp.tile([128, 1], F32)
            nc.vector.memset(eps_col[:], EPS)
            zero_i = cp.tile([1, 1], I32)
            nc.vector.memset(zero_i[:], 0)
            one_u8 = cp.tile([1, 1], U8)
            nc.vector.memset(one_u8[:], 1)
            dumm = wp.tile([1, 1], F32, tag="dumm")
            nc.scalar.square(dumm[:], one_cell[:])
            nc.scalar.activation(dumm[:], one_cell[:], mybir.ActivationFunctionType.Sqrt)
            nc.scalar.activation(dumm[:], one_cell[:], mybir.ActivationFunctionType.Ln,
                                 bias=eps_col[0:1, 0:1], scale=1.0)

            # ---- static work that depends on nothing: cls row of new_attn ----
            g0 = gp.tile([H, N], F32, tag="g0")
            nc.scalar.dma_start(out=g0[:], in_=bass.AP(attn_d, 0, [[N * N, H], [1, N]]))
            nc.scalar.dma_start(out=oattn_d[:][:, 0, :], in_=g0[:])
            nc.sync.dma_start(out=oids_d[:][0:1, None], in_=zero_i[:])
            nc.sync.dma_start(out=omask_d[:][0:1, None], in_=one_u8[:])

            # ---- constants (scalar queue) ----
            ident = cp.tile([128, 128], F32)
            nc.scalar.dma_start(out=ident[:], in_=ident_c[:])
            iota128 = cp.tile([128, 128], F32)
            nc.scalar.dma_start(out=iota128[:], in_=iota128_c[:])
            ilt, iltr = [], []
            for a in range(2):
                t = cp.tile([128, K], F32, tag=f"ilt{a}")
                nc.scalar.dma_start(out=t[:], in_=ilt_c[a][:])
                ilt.append(t)
                t2 = cp.tile([128, K], F32, tag=f"iltr{a}")
                nc.scalar.dma_start(out=t2[:], in_=iltr_c[a][:])
                iltr.append(t2)
            hoffm = cp.tile([128, 2 * H], F32)
            nc.scalar.dma_start(out=hoffm[:], in_=hoffm_c[:])

            # PE warmup: observe const DMAs once so PE-transposes need only a DVE wait
            warm = ps.tile([1, 1], F32, tag="small")
            nc.tensor.matmul(warm[:], lhsT=ident[:, 0:1], rhs=iota128[:, 0:1],
                             start=True, stop=True)

            # ---- gumbel preload (scalar queue; independent of everything) ----
            gts = []
            for a in range(2):
                gt = kp.tile([128, NM1], F32, tag=f"gt{a}")
                nc.scalar.dma_start(out=gt[:], in_=gum_d[:][a * 128:(a + 1) * 128, :])
                gts.append(gt)

            # ---- stage 1: scores (token-partition layout: token j-1 = 8p + c) ----
            norms2 = kp.tile([128, H * 8], F32)
            for k in range(H // VCH):
                h0 = k * VCH
                vt = wp.tile([128, VCH * 512], F32, tag="vt")
                nc.sync.dma_start(
                    out=vt[:].rearrange("p (h f) -> p h f", f=512),
                    in_=val_d[:][h0:h0 + VCH, D:].rearrange("h (p f) -> p h f", f=512),
                )
                sqc = wp.tile([128, VCH * 512], F32, tag="sqc")
                nc.scalar.square(sqc[:], vt[:])
                nc.vector.tensor_reduce(
                    out=norms2[:, h0 * 8:(h0 + VCH) * 8],
                    in_=sqc[:].rearrange("p (g d) -> p g d", d=D),
                    axis=mybir.AxisListType.X, op=ALU.add)
            norms = kp.tile([128, H * 8], F32)
            nc.scalar.sqrt(norms[:], norms2[:])

            cls = kp.tile([128, H * 8], F32)
            nc.sync.dma_start(
                out=cls[:].rearrange("p (h c) -> p h c", c=8),
                in_=bass.AP(attn_d, 1, [[8, 128], [N * N, H], [1, 8]]),
            )
            prod = kp.tile([128, H * 8], F32)
            nc.vector.tensor_mul(prod[:], cls[:], norms[:])
            score = kp.tile([128, 8], F32)
            nc.vector.tensor_reduce(
                out=score[:], in_=prod[:].rearrange("p (h c) -> p c h", c=8),
                axis=mybir.AxisListType.X, op=ALU.add)

            sumrow = wp.tile([128, 1], F32)
            nc.vector.tensor_reduce(out=sumrow[:], in_=score[:],
                                    axis=mybir.AxisListType.X, op=ALU.add)
            total_ps = ps.tile([1, 1], F32, tag="small")
            nc.tensor.matmul(total_ps[:], lhsT=sumrow[:], rhs=ones_col[:],
                             start=True, stop=True)
            total = wp.tile([1, 1], F32)
            nc.vector.tensor_scalar(total[:], total_ps[:], EPS, None, op0=ALU.add)
            recip = wp.tile([1, 1], F32)
            nc.vector.reciprocal(recip[:], total[:])
            # broadcast recip to 128 partitions: K=1 matmul with a ones row (exact: 1.0 * recip)
            recB_ps = ps.tile([128, 1], F32, tag="small")
            nc.tensor.matmul(recB_ps[:], lhsT=ones_row[:], rhs=recip[:],
                             start=True, stop=True)
            recipB = wp.tile([128, 1], F32)
            nc.vector.tensor_copy(recipB[:], recB_ps[:])

            pl = kp.tile([128, 8], F32)
            nc.scalar.activation(pl[:], score[:], mybir.ActivationFunctionType.Ln,
                                 bias=eps_col[:, 0:1], scale=recipB[:, 0:1])
            # mask (all ones in practice; exact reference semantics)
            mku = wp.tile([128, 8], U8)
            nc.sync.dma_start(out=mku[:], in_=msk_d[:][None, 1:].rearrange("o (p c) -> (o p) c", c=8))
            mkf = wp.tile([128, 8], F32)
            nc.vector.tensor_copy(mkf[:], mku[:])
            plm = kp.tile([128, 8], F32)
            nc.vector.tensor_mul(plm[:], pl[:], mkf[:])
            inv = wp.tile([128, 8], F32)
            nc.vector.tensor_scalar(inv[:], mkf[:], 0.5, MASKVAL, op0=ALU.is_lt, op1=ALU.mult)
            nc.vector.tensor_add(plm[:], plm[:], inv[:])
            nc.sync.dma_start(out=pl_dram[:].rearrange("(p c) -> p c", c=8), in_=plm[:])

            # ---- stage 2: gumbel argmax sampling ----
            ids_col = []
            for a in range(2):
                gt = gts[a]
                nc.gpsimd.dma_start(out=gt[:], in_=bass.AP(pl_dram, 0, [[0, 128], [1, NM1]]),
                                    accum_op=ALU.add)
                mx8 = wp.tile([128, 8], F32, tag="mx8")
                nc.vector.max(out=mx8[:], in_=gt[:])
                ix8 = wp.tile([128, 8], U32, tag="ix8")
                nc.vector.max_index(out=ix8[:], in_max=mx8[:], in_values=gt[:])
                idc = kp.tile([128, 1], F32, tag=f"idc{a}")
                nc.vector.tensor_scalar(idc[:], ix8[:, 0:1], 1.0, None, op0=ALU.add)
                ids_col.append(idc)

            # ---- stage 3: unique + sorted positions (exact small-int arithmetic in f32) ----
            idsT = kp.tile([128, K], F32)
            for a in range(2):
                tp = ps.tile([128, 128], F32, tag="tp")
                nc.tensor.transpose(tp[:], ids_col[a][:].to_broadcast([128, 128]), ident[:])
                nc.vector.tensor_copy(idsT[:, a * 128:(a + 1) * 128], tp[:])

            LT, F_col = [], []
            for a in range(2):
                lt = kp.tile([128, K], F32, tag=f"lt{a}")
                nc.vector.tensor_tensor(lt[:], ids_col[a][:].to_broadcast([128, K]), idsT[:],
                                        op=ALU.is_lt)
                eq = wp.tile([128, K], F32, tag="eq")
                nc.vector.tensor_tensor(eq[:], ids_col[a][:].to_broadcast([128, K]), idsT[:],
                                        op=ALU.is_equal)
                nc.vector.tensor_mul(eq[:], eq[:], iltr[a][:])
                ec = wp.tile([128, 1], F32, tag="ec")
                nc.vector.tensor_reduce(out=ec[:], in_=eq[:],
                                        axis=mybir.AxisListType.X, op=ALU.add)
                fc = kp.tile([128, 1], F32, tag=f"fc{a}")
                nc.vector.tensor_scalar(fc[:], ec[:], 0.0, None, op0=ALU.is_equal)
                LT.append(lt)
                F_col.append(fc)

            pos_ps = psa.tile([1, K], F32, tag="acc")
            for a in range(2):
                nc.tensor.matmul(pos_ps[:], lhsT=F_col[a][:], rhs=LT[a][:],
                                 start=(a == 0), stop=(a == 1))
            pos = kp.tile([1, K], F32)
            nc.vector.tensor_copy(pos[:], pos_ps[:])

            OH, RHS = [], []
            for a in range(2):
                pcp = ps.tile([128, 1], F32, tag="small")
                nc.tensor.matmul(pcp[:], lhsT=pos[0:1, a * 128:(a + 1) * 128], rhs=one_cell[:],
                                 start=True, stop=True)
                # q' = pos (0-based over the 256 non-cls slots) for survivors, DUMP otherwise
                nsv = wp.tile([128, 1], F32, tag="nsv")
                nc.vector.tensor_scalar(nsv[:], F_col[a][:], 0.5, DUMP, op0=ALU.is_lt, op1=ALU.mult)
                p1 = wp.tile([128, 1], F32, tag="p1")
                nc.vector.tensor_scalar(p1[:], pcp[:], F_col[a][:, 0:1], None, op0=ALU.mult)
                nc.vector.tensor_add(p1[:], p1[:], nsv[:])
                d1 = wp.tile([128, 1], F32, tag="d1")
                nc.vector.tensor_scalar(d1[:], p1[:], 128.0, None, op0=ALU.is_ge)
                d2 = wp.tile([128, 1], F32, tag="d2")
                nc.vector.tensor_scalar(d2[:], p1[:], 256.0, None, op0=ALU.is_ge)
                dvs = wp.tile([128, 1], F32, tag="dvs")
                nc.vector.tensor_add(dvs[:], d1[:], d2[:])
                nc.vector.tensor_scalar(dvs[:], dvs[:], 128.0, None, op0=ALU.mult)
                md = wp.tile([128, 1], F32, tag="md")
                nc.vector.tensor_sub(md[:], p1[:], dvs[:])
                oh = kp.tile([128, 128], F32, tag=f"oh{a}")
                nc.vector.tensor_tensor(oh[:], md[:].to_broadcast([128, 128]), iota128[:],
                                        op=ALU.is_equal)
                dv0 = wp.tile([128, 1], F32, tag="dv0")
                nc.vector.tensor_scalar(dv0[:], p1[:], 128.0, None, op0=ALU.is_lt)
                dv1 = wp.tile([128, 1], F32, tag="dv1")
                nc.vector.tensor_sub(dv1[:], d1[:], d2[:])
                rhs = kp.tile([128, 2], F32, tag=f"rhs{a}")
                nc.vector.tensor_mul(rhs[:, 0:1], ids_col[a][:], dv0[:])
                nc.vector.tensor_mul(rhs[:, 1:2], ids_col[a][:], dv1[:])
                OH.append(oh)
                RHS.append(rhs)

            vals_ps = psa.tile([128, 2], F32, tag="acc")
            for a in range(2):
                nc.tensor.matmul(vals_ps[:], lhsT=OH[a][:], rhs=RHS[a][:],
                                 start=(a == 0), stop=(a == 1))
            vals = kp.tile([128, 2], F32)  # (x, c) -> unique_ids[1 + 128c + x]
            nc.vector.tensor_copy(vals[:], vals_ps[:])

            # ---- stage 4: ids / mask outputs ----
            idsi = kp.tile([128, 2], I32)
            nc.vector.tensor_copy(idsi[:], vals[:])
            nc.sync.dma_start(out=oids_d[:][1:129, None], in_=idsi[:, 0:1])
            nc.sync.dma_start(out=oids_d[:][129:257, None], in_=idsi[:, 1:2])
            mcol = kp.tile([128, 2], U8)
            nc.vector.tensor_scalar(mcol[:], vals[:], 0.0, None, op0=ALU.not_equal)
            nc.sync.dma_start(out=omask_d[:][1:129, None], in_=mcol[:, 0:1])
            nc.sync.dma_start(out=omask_d[:][129:257, None], in_=mcol[:, 1:2])

            # ---- stage 5: gather attn rows (24 x [128 rows]) ----
            idxf = kp.tile([128, 2 * H], F32)
            vals_b = bass.AP(vals[:].tensor, vals[:].offset,
                             [vals[:].ap[0], [0, H], vals[:].ap[1]])
            nc.vector.tensor_tensor(idxf[:].rearrange("p (h c) -> p h c", c=2),
                                    vals_b, hoffm[:].rearrange("p (h c) -> p h c", c=2),
                                    op=ALU.add)
            idxi = kp.tile([128, 2 * H], I32)
            nc.vector.tensor_copy(idxi[:], idxf[:])

            wengs = (nc.sync, nc.scalar)
            i = 0
            for h in range(H):
                for c in range(2):
                    g = gp.tile([128, N], F32, tag="g")
                    _indirect_gather_q(nc, g[:], attn_d[:],
                                       idxi[:, 2 * h + c:2 * h + c + 1], 0, i % 4)
                    wengs[i % 2].dma_start(
                        out=oattn_d[:][h, 1 + c * 128:1 + (c + 1) * 128, :], in_=g[:])
                    i += 1

    nc.finalize()
    return nc


_NC = None


def _get_nc():
    global _NC
    if _NC is None:
        _NC = _build()
    return _NC


def _run(attn, value, mask, gumbel, trace=False):
    attn = np.ascontiguousarray(np.asarray(attn, dtype=np.float32))
    value = np.ascontiguousarray(np.asarray(value, dtype=np.float32))
    gumbel = np.ascontiguousarray(np.asarray(gumbel, dtype=np.float32))
    mask_u8 = np.ascontiguousarray(np.asarray(mask).astype(np.uint8))

    in_maps = [
        {
            "attn": attn[b].reshape(NH, N),
            "value": value[b].reshape(H, N * D),
            "maskp": mask_u8[b],
            "gumbel": gumbel[b],
        }
        for b in range(B)
    ]
    nc = _get_nc()
    res = run_bass_kernel_spmd(nc, in_maps, list(range(B)), trace=trace)

    new_attn = np.stack([np.asarray(res.results[b]["out_attn"]) for b in range(B)])
    unique_ids = np.stack([np.asarray(res.results[b]["out_ids"]) for b in range(B)])
    new_mask = np.stack([np.asarray(res.results[b]["out_mask"]) for b in range(B)]).astype(bool)
    return (new_attn, new_mask, unique_ids.astype(np.int32)), res


def kernel(attn, value, mask, gumbel):
    out, _ = _run(attn, value, mask, gumbel, trace=False)
    return out
